# revision 29
# baseline (speedup 1.0000x reference)
"""GQA attention layer (QK-RMSNorm + RoPE + causal GQA + o_proj) on 8 TRN2 cores.

Sharding: tensor-parallel over heads. Core c owns Q heads [4c..4c+3] and KV
head c (one full GQA group), full sequence. Every core runs the identical
program (SPMD), differing only in its weight shards. o_proj partial sums are
reduced on the host; K/V outputs are concatenated per kv-head on the host.
"""

import sys

for _p in ("/opt/trn_rl_repo", "/opt/pypackages"):
    if _p not in sys.path:
        sys.path.append(_p)

import numpy as np
import ml_dtypes

import concourse.bass as bass
import concourse.tile as tile
from concourse import bacc, mybir
from concourse import bass_utils
from concourse.masks import make_identity

BF16 = ml_dtypes.bfloat16

# Problem constants (hardcoded per harness contract)
B, S, HID = 2, 2048, 2560
H, KV, D = 32, 8, 128
G = H // KV  # 4 query heads per kv head
EPS = 1e-6
NCORES = 8
HD_PER_CORE = G * D  # 512 q-head channels per core
T = B * S  # 4096 tokens total
TB = S  # tokens per batch
NT = TB // 128  # 16 token tiles per batch
NC = HID // 128  # 20 contraction tiles
SCALE = 1.0 / np.sqrt(D)

_COMPILED = None


def _build():
    """Build + compile the SPMD Bass graph (once)."""
    nc = bacc.Bacc("TRN2", target_bir_lowering=False, debug=False,
                   enable_asserts=True, num_devices=NCORES)
    f32 = mybir.dt.float32
    bf16 = mybir.dt.bfloat16

    # --- per-core DRAM I/O ---
    hid_t = nc.dram_tensor("hid_t", [HID, T], bf16, kind="ExternalInput").ap()
    wq_t = nc.dram_tensor("wq_t", [HID, HD_PER_CORE], bf16, kind="ExternalInput").ap()
    wkv_t = nc.dram_tensor("wkv_t", [HID, 2 * D], bf16, kind="ExternalInput").ap()
    wo_t = nc.dram_tensor("wo_t", [HD_PER_CORE, HID], bf16, kind="ExternalInput").ap()
    cos_q = nc.dram_tensor("cos_q", [T, D], bf16, kind="ExternalInput").ap()
    sin_q = nc.dram_tensor("sin_q", [T, D], bf16, kind="ExternalInput").ap()
    cos_k = nc.dram_tensor("cos_k", [T, D], bf16, kind="ExternalInput").ap()
    sin_k = nc.dram_tensor("sin_k", [T, D], bf16, kind="ExternalInput").ap()

    out_p = nc.dram_tensor("out_p", [T, HID], bf16, kind="ExternalOutput").ap()
    knew = nc.dram_tensor("knew", [T, D], bf16, kind="ExternalOutput").ap()
    vnew = nc.dram_tensor("vnew", [T, D], bf16, kind="ExternalOutput").ap()

    # DRAM views tiled for 128-partition DMA
    hid_v = hid_t.rearrange("(n p) t -> p n t", p=128)       # [128, NC, T]
    wq_v = wq_t.rearrange("(n p) h -> p n h", p=128)         # [128, NC, 512]
    wkv_v = wkv_t.rearrange("(n p) h -> p n h", p=128)       # [128, NC, 256]
    wo_v = wo_t.rearrange("(n p) e -> p n e", p=128)         # [128, 4, HID]

    from contextlib import ExitStack
    with tile.TileContext(nc) as tc, ExitStack() as es:
        cpool = es.enter_context(tc.tile_pool(name="const", bufs=1))
        wpool = es.enter_context(tc.tile_pool(name="weights", bufs=1))
        hpool = es.enter_context(tc.tile_pool(name="hid", bufs=4))
        rpool = es.enter_context(tc.tile_pool(name="rope", bufs=1))
        qkv = es.enter_context(tc.tile_pool(name="qkv", bufs=2))
        spool = es.enter_context(tc.tile_pool(name="small", bufs=4))
        ppool = es.enter_context(tc.tile_pool(name="probs", bufs=6))
        apool = es.enter_context(tc.tile_pool(name="attnT", bufs=1))
        psA = es.enter_context(tc.tile_pool(name="psA", bufs=3, space="PSUM"))
        psB = es.enter_context(tc.tile_pool(name="psB", bufs=2, space="PSUM"))
        psC = es.enter_context(tc.tile_pool(name="psC", bufs=3, space="PSUM"))

        ident = cpool.tile([128, 128], bf16)
        make_identity(nc, ident)
        ones_col = cpool.tile([128, 1], bf16)
        nc.gpsimd.memset(ones_col, 1.0)
        zero_s = cpool.tile([128, 1], f32)
        nc.gpsimd.memset(zero_s, 0.0)
        eps_s = cpool.tile([128, 1], f32)
        nc.gpsimd.memset(eps_s, float(EPS))

        # resident weights
        wq_sb = wpool.tile([128, NC, HD_PER_CORE], bf16, tag="wq")
        wkv_sb = wpool.tile([128, NC, 2 * D], bf16, tag="wkv")
        wo_sb = wpool.tile([128, G, HID], bf16, tag="wo")
        for _c in range(4):
            nc.sync.dma_start(out=wq_sb[:, _c * 5:(_c + 1) * 5, :],
                              in_=wq_v[:, _c * 5:(_c + 1) * 5, :])
        nc.sync.dma_start(out=wkv_sb, in_=wkv_v)

        # PE warmup: ~8us of junk matmuls overlapping the initial weight DMA,
        # so the HAM clock-gate is open when real matmuls start
        warm_in = cpool.tile([128, 512], bf16, tag="warm")
        nc.gpsimd.memset(warm_in, 0.0)
        warm_ps = psA.tile([128, 512], f32, tag="mm512")
        for _w in range(150):
            nc.tensor.matmul(warm_ps, warm_in[:, 0:128], warm_in,
                             start=(_w == 0), stop=(_w == 149))
        wo_loaded = [False]

        for b in range(B):
            t0b = b * TB

            # rope tables for this batch (resident, [128, NT, D]);
            # DMAs are emitted inside the ti==0 body, after the projection
            # matmuls, so they don't delay the first matmul's inputs
            cq_sb = rpool.tile([128, NT, D], bf16, tag="cq")
            sq_sb = rpool.tile([128, NT, D], bf16, tag="sq")
            ck_sb = rpool.tile([128, NT, D], bf16, tag="ck")
            sk_sb = rpool.tile([128, NT, D], bf16, tag="sk")

            def load_rope_tables():
                for ap_sb, ap_dr in ((cq_sb, cos_q), (sq_sb, sin_q),
                                     (ck_sb, cos_k), (sk_sb, sin_k)):
                    nc.sync.dma_start(
                        out=ap_sb,
                        in_=ap_dr[t0b:t0b + TB, :].rearrange("(n p) d -> p n d", p=128))

            # per-batch activation stores
            qt_sb = qkv.tile([128, G, TB], bf16, tag="qt")    # [d, h, t] Q^T
            kt_sb = qkv.tile([128, TB], bf16, tag="kt")       # [d, t]    K^T
            v_sb = qkv.tile([128, NT, D], bf16, tag="v")      # [t_p, ti, d]
            at_sb = apool.tile([128, G, TB], bf16, tag="at")  # [d, h, t] attnT

            # ---- phase 1: projections + RMSNorm + RoPE + transposes ----
            for ti in range(NT):
                t0 = t0b + ti * 128
                hid_sb = hpool.tile([128, NC, 128], bf16, tag="hid")
                nc.sync.dma_start(out=hid_sb, in_=hid_v[:, :, t0:t0 + 128])

                q_ps = psA.tile([128, HD_PER_CORE], f32, tag="mm512")
                kv_ps = psB.tile([128, 2 * D], f32, tag="kvsum")
                k_ps = kv_ps[:, 0:D]
                v_ps = kv_ps[:, D:2 * D]
                for n in range(NC):
                    st, sp = (n == 0), (n == NC - 1)
                    nc.tensor.matmul(q_ps, hid_sb[:, n, :], wq_sb[:, n, :], start=st, stop=sp)
                for n in range(NC):
                    st, sp = (n == 0), (n == NC - 1)
                    nc.tensor.matmul(kv_ps, hid_sb[:, n, :], wkv_sb[:, n, :], start=st, stop=sp)

                if ti == 0:
                    load_rope_tables()

                # RMSNorm stats: ssq per head -> one batched sqrt+recip
                sq_scratch = spool.tile([128, 128], bf16, tag="sqs")
                ssq_all = spool.tile([128, G + 1], f32, tag="ssq")
                for h in range(G):
                    nc.scalar.activation(sq_scratch, q_ps[:, h * D:(h + 1) * D],
                                         mybir.ActivationFunctionType.Square,
                                         bias=zero_s[:, 0:1],
                                         accum_out=ssq_all[:, h:h + 1])
                nc.scalar.activation(sq_scratch, k_ps,
                                     mybir.ActivationFunctionType.Square,
                                     bias=zero_s[:, 0:1],
                                     accum_out=ssq_all[:, G:G + 1])
                t1a = spool.tile([128, G + 1], f32, tag="t1a")
                nc.scalar.activation(t1a, ssq_all, mybir.ActivationFunctionType.Sqrt,
                                     bias=eps_s[:, 0:1], scale=1.0 / D)
                rstd_all = spool.tile([128, G + 1], f32, tag="rsall")
                nc.vector.reciprocal_approx_fast(rstd_all, t1a)
                neg = spool.tile([128, G + 1], f32, tag="neg")
                nc.vector.tensor_scalar_mul(neg, rstd_all, -1.0)
                rstd_q = [rstd_all[:, h:h + 1] for h in range(G)]
                rs_k = rstd_all[:, G:G + 1]

                Dh = D // 2
                mul = mybir.AluOpType.mult

                def rope(src_ps, rstd, nrstd, cos_t, sin_t, out_bf):
                    a = spool.tile([128, D], bf16, tag="ra")
                    bb = spool.tile([128, D], bf16, tag="rb")
                    nc.vector.scalar_tensor_tensor(a, src_ps, rstd, cos_t, mul, mul)
                    nc.vector.scalar_tensor_tensor(bb[:, 0:Dh], src_ps[:, Dh:D],
                                                   nrstd, sin_t[:, 0:Dh], mul, mul)
                    nc.vector.scalar_tensor_tensor(bb[:, Dh:D], src_ps[:, 0:Dh],
                                                   rstd, sin_t[:, Dh:D], mul, mul)
                    nc.vector.tensor_add(out_bf, a, bb)

                qr = spool.tile([128, HD_PER_CORE], bf16, tag="qr")
                for h in range(G):
                    rope(q_ps[:, h * D:(h + 1) * D], rstd_q[h], neg[:, h:h + 1],
                         cq_sb[:, ti, :], sq_sb[:, ti, :], qr[:, h * D:(h + 1) * D])
                kr = spool.tile([128, D], bf16, tag="kr")
                rope(k_ps, rs_k, neg[:, G:G + 1], ck_sb[:, ti, :], sk_sb[:, ti, :], kr)

                # V: psum -> sbuf bf16 (both for AV lhsT and for output)
                nc.vector.tensor_copy(v_sb[:, ti, :], v_ps)
                nc.sync.dma_start(out=vnew[t0:t0 + 128, :], in_=v_sb[:, ti, :])
                nc.sync.dma_start(out=knew[t0:t0 + 128, :], in_=kr)

                # transposes into [d, t] layout on PE
                # (1/sqrt(D) scale is pre-folded into cos_q/sin_q on host)
                tp = psC.tile([128, 5 * 128], bf16, tag="accT")
                for h in range(G):
                    nc.tensor.transpose(tp[:, h * 128:(h + 1) * 128],
                                        qr[:, h * D:(h + 1) * D], ident)
                nc.tensor.transpose(tp[:, 4 * 128:5 * 128], kr, ident)
                nc.vector.tensor_copy(
                    qt_sb[:, :, ti * 128:(ti + 1) * 128],
                    tp[:, 0:4 * 128].rearrange("p (g t) -> p g t", g=G))
                nc.vector.tensor_copy(kt_sb[:, ti * 128:(ti + 1) * 128],
                                      tp[:, 4 * 128:5 * 128])

            if not wo_loaded[0]:
                nc.sync.dma_start(out=wo_sb, in_=wo_v)
                wo_loaded[0] = True

            # ---- phase 2: attention (transposed layout, exact causal) ----
            for h in range(G):
                for ci in range(4):  # query chunks of 512
                    nl = 4 * (ci + 1)  # causal l-tiles of 128
                    q_sl = qt_sb[:, h, ci * 512:(ci + 1) * 512]
                    sums = psB.tile([1, 512], f32, tag="kvsum")
                    acc = psC.tile([128, 512], f32, tag="accT")
                    for lt in range(nl):
                        # diagonal band: trim query columns that precede key lt
                        qs = max(0, (lt - 4 * ci) * 128)
                        n = 512 - qs
                        sc = psA.tile([128, 512], f32, tag="mm512")
                        nc.tensor.matmul(sc[:, 0:n], kt_sb[:, lt * 128:(lt + 1) * 128],
                                         q_sl[:, qs:512], start=True, stop=True)
                        pt = ppool.tile([128, 512], bf16, tag="pt")
                        nc.scalar.activation(pt[:, 0:n], sc[:, 0:n],
                                             mybir.ActivationFunctionType.Exp,
                                             bias=zero_s[:, 0:1])
                        if lt >= 4 * ci:  # zero strictly-future queries in diag tile
                            nc.gpsimd.affine_select(
                                pt[:, 0:n], pt[:, 0:n], pattern=[[1, n]],
                                compare_op=mybir.AluOpType.is_ge, fill=0.0,
                                base=0, channel_multiplier=-1)
                        st, sp = (lt == 0), (lt == nl - 1)
                        nc.tensor.matmul(sums[:, qs:512], ones_col, pt[:, 0:n],
                                         start=st, stop=sp)
                        nc.tensor.matmul(acc[:, qs:512], v_sb[:, lt, :], pt[:, 0:n],
                                         start=st, stop=sp)
                    rec = spool.tile([1, 512], f32, tag="rec")
                    nc.vector.reciprocal_approx_fast(rec, sums)
                    rec_bc = spool.tile([128, 512], f32, tag="recbc")
                    nc.gpsimd.partition_broadcast(rec_bc, rec)
                    nc.vector.tensor_mul(at_sb[:, h, ci * 512:(ci + 1) * 512],
                                         acc, rec_bc)

            # ---- phase 3: o_proj partials, DMA straight from PSUM ----
            for ti in range(NT):
                t0 = t0b + ti * 128
                for ec in range(HID // 512):
                    op = psA.tile([128, 512], f32, tag="mm512")
                    for h in range(G):
                        nc.tensor.matmul(op, at_sb[:, h, ti * 128:(ti + 1) * 128],
                                         wo_sb[:, h, ec * 512:(ec + 1) * 512],
                                         start=(h == 0), stop=(h == G - 1))
                    ob = ppool.tile([128, 512], bf16, tag="ob")
                    nc.vector.tensor_copy(ob, op)
                    nc.sync.dma_start(out=out_p[t0:t0 + 128, ec * 512:(ec + 1) * 512],
                                      in_=ob)

    nc.compile()
    return nc


def kernel(hidden, cos, sin, wq, wk, wv, wo, q_gamma, k_gamma):
    global _COMPILED
    if _COMPILED is None:
        _COMPILED = _build()
    nc = _COMPILED

    hid_t = np.ascontiguousarray(hidden.reshape(T, HID).T).astype(BF16)
    wq_t = wq.T.astype(BF16)   # (HID, H*D)
    wk_t = wk.T.astype(BF16)
    wv_t = wv.T.astype(BF16)
    # merged K|V projection weights, per-core slice stacked on output dim
    wo_t = np.ascontiguousarray(wo.T)  # (H*D, HID)

    cos2 = cos.reshape(T, D).astype(np.float32)
    sin2 = sin.reshape(T, D).astype(np.float32)
    qg = q_gamma.astype(np.float32)
    kg = k_gamma.astype(np.float32)
    qg_sw = np.concatenate([qg[D // 2:], qg[:D // 2]])
    kg_sw = np.concatenate([kg[D // 2:], kg[:D // 2]])
    cos_q = (cos2 * qg * SCALE).astype(BF16)
    sin_q = (sin2 * qg_sw * SCALE).astype(BF16)
    cos_k = (cos2 * kg).astype(BF16)
    sin_k = (sin2 * kg_sw).astype(BF16)

    in_maps = []
    for c in range(NCORES):
        in_maps.append({
            "hid_t": hid_t,
            "wq_t": np.ascontiguousarray(wq_t[:, c * HD_PER_CORE:(c + 1) * HD_PER_CORE]),
            "wkv_t": np.ascontiguousarray(np.concatenate(
                [wk_t[:, c * D:(c + 1) * D], wv_t[:, c * D:(c + 1) * D]], axis=1)),
            "wo_t": np.ascontiguousarray(wo_t[c * HD_PER_CORE:(c + 1) * HD_PER_CORE, :]).astype(BF16),
            "cos_q": cos_q, "sin_q": sin_q, "cos_k": cos_k, "sin_k": sin_k,
        })

    res = bass_utils.run_bass_kernel_spmd(nc, in_maps, core_ids=list(range(NCORES)))
    outs = res.results

    out = np.zeros((T, HID), dtype=np.float32)
    for c in range(NCORES):
        out += np.asarray(outs[c]["out_p"], dtype=np.float32)
    out = out.reshape(B, S, HID)

    new_k = np.stack(
        [np.asarray(outs[c]["knew"]).astype(np.float32).reshape(B, S, D)
         for c in range(NCORES)], axis=1)
    new_v = np.stack(
        [np.asarray(outs[c]["vnew"]).astype(np.float32).reshape(B, S, D)
         for c in range(NCORES)], axis=1)
    return out, new_k, new_v


# revision 31
# speedup vs baseline: 1.0140x; 1.0140x over previous
"""GQA attention layer (QK-RMSNorm + RoPE + causal GQA + o_proj) on 8 TRN2 cores.

Sharding: tensor-parallel over heads. Core c owns Q heads [4c..4c+3] and KV
head c (one full GQA group), full sequence. Every core runs the identical
program (SPMD), differing only in its weight shards. o_proj partial sums are
reduced on the host; K/V outputs are concatenated per kv-head on the host.
"""

import sys

for _p in ("/opt/trn_rl_repo", "/opt/pypackages"):
    if _p not in sys.path:
        sys.path.append(_p)

import numpy as np
import ml_dtypes

import concourse.bass as bass
import concourse.tile as tile
from concourse import bacc, mybir
from concourse import bass_utils
from concourse.masks import make_identity

BF16 = ml_dtypes.bfloat16

# Problem constants (hardcoded per harness contract)
B, S, HID = 2, 2048, 2560
H, KV, D = 32, 8, 128
G = H // KV  # 4 query heads per kv head
EPS = 1e-6
NCORES = 8
HD_PER_CORE = G * D  # 512 q-head channels per core
T = B * S  # 4096 tokens total
TB = S  # tokens per batch
NT = TB // 128  # 16 token tiles per batch
NC = HID // 128  # 20 contraction tiles
SCALE = 1.0 / np.sqrt(D)

_COMPILED = None


def _build():
    """Build + compile the SPMD Bass graph (once)."""
    nc = bacc.Bacc("TRN2", target_bir_lowering=False, debug=False,
                   enable_asserts=True, num_devices=NCORES)
    f32 = mybir.dt.float32
    bf16 = mybir.dt.bfloat16

    # --- per-core DRAM I/O ---
    hid_t = nc.dram_tensor("hid_t", [HID, T], bf16, kind="ExternalInput").ap()
    wq_t = nc.dram_tensor("wq_t", [HID, HD_PER_CORE], bf16, kind="ExternalInput").ap()
    wkv_t = nc.dram_tensor("wkv_t", [HID, 2 * D], bf16, kind="ExternalInput").ap()
    wo_t = nc.dram_tensor("wo_t", [HD_PER_CORE, HID], bf16, kind="ExternalInput").ap()
    cos_q = nc.dram_tensor("cos_q", [T, D], bf16, kind="ExternalInput").ap()
    sin_q = nc.dram_tensor("sin_q", [T, D], bf16, kind="ExternalInput").ap()
    cos_k = nc.dram_tensor("cos_k", [T, D], bf16, kind="ExternalInput").ap()
    sin_k = nc.dram_tensor("sin_k", [T, D], bf16, kind="ExternalInput").ap()

    out_p = nc.dram_tensor("out_p", [T, HID], bf16, kind="ExternalOutput").ap()
    knew = nc.dram_tensor("knew", [T, D], bf16, kind="ExternalOutput").ap()
    vnew = nc.dram_tensor("vnew", [T, D], bf16, kind="ExternalOutput").ap()

    # DRAM views tiled for 128-partition DMA
    hid_v = hid_t.rearrange("(n p) t -> p n t", p=128)       # [128, NC, T]
    wq_v = wq_t.rearrange("(n p) h -> p n h", p=128)         # [128, NC, 512]
    wkv_v = wkv_t.rearrange("(n p) h -> p n h", p=128)       # [128, NC, 256]
    wo_v = wo_t.rearrange("(n p) e -> p n e", p=128)         # [128, 4, HID]

    from contextlib import ExitStack
    with tile.TileContext(nc) as tc, ExitStack() as es:
        cpool = es.enter_context(tc.tile_pool(name="const", bufs=1))
        wpool = es.enter_context(tc.tile_pool(name="weights", bufs=1))
        hpool = es.enter_context(tc.tile_pool(name="hid", bufs=3))
        rpool = es.enter_context(tc.tile_pool(name="rope", bufs=1))
        qkv = es.enter_context(tc.tile_pool(name="qkv", bufs=2))
        spool = es.enter_context(tc.tile_pool(name="small", bufs=4))
        ppool = es.enter_context(tc.tile_pool(name="probs", bufs=6))
        apool = es.enter_context(tc.tile_pool(name="attnT", bufs=1))
        psA = es.enter_context(tc.tile_pool(name="psA", bufs=4, space="PSUM"))
        psB = es.enter_context(tc.tile_pool(name="psB", bufs=2, space="PSUM"))
        psC = es.enter_context(tc.tile_pool(name="psC", bufs=2, space="PSUM"))

        ident = cpool.tile([128, 128], bf16)
        make_identity(nc, ident)
        ones_col = cpool.tile([128, 1], bf16)
        nc.gpsimd.memset(ones_col, 1.0)
        zero_s = cpool.tile([128, 1], f32)
        nc.gpsimd.memset(zero_s, 0.0)
        eps_s = cpool.tile([128, 1], f32)
        nc.gpsimd.memset(eps_s, float(EPS))

        # resident weights
        wq_sb = wpool.tile([128, NC, HD_PER_CORE], bf16, tag="wq")
        wkv_sb = wpool.tile([128, NC, 2 * D], bf16, tag="wkv")
        wo_sb = wpool.tile([128, G, HID], bf16, tag="wo")
        for _c in range(4):
            nc.sync.dma_start(out=wq_sb[:, _c * 5:(_c + 1) * 5, :],
                              in_=wq_v[:, _c * 5:(_c + 1) * 5, :])
        nc.sync.dma_start(out=wkv_sb, in_=wkv_v)

        # PE warmup: ~8us of junk matmuls overlapping the initial weight DMA,
        # so the HAM clock-gate is open when real matmuls start
        warm_in = cpool.tile([128, 512], bf16, tag="warm")
        nc.gpsimd.memset(warm_in, 0.0)
        warm_ps = psA.tile([128, 512], f32, tag="mm512")
        for _w in range(100):
            nc.tensor.matmul(warm_ps, warm_in[:, 0:128], warm_in,
                             start=(_w == 0), stop=(_w == 99))
        wo_loaded = [False]

        for b in range(B):
            t0b = b * TB

            # rope tables for this batch (resident, [128, NT, D]);
            # DMAs are emitted inside the ti==0 body, after the projection
            # matmuls, so they don't delay the first matmul's inputs
            cq_sb = rpool.tile([128, NT, D], bf16, tag="cq")
            sq_sb = rpool.tile([128, NT, D], bf16, tag="sq")
            ck_sb = rpool.tile([128, NT, D], bf16, tag="ck")
            sk_sb = rpool.tile([128, NT, D], bf16, tag="sk")

            def load_rope_tables():
                for ap_sb, ap_dr in ((cq_sb, cos_q), (sq_sb, sin_q),
                                     (ck_sb, cos_k), (sk_sb, sin_k)):
                    nc.sync.dma_start(
                        out=ap_sb,
                        in_=ap_dr[t0b:t0b + TB, :].rearrange("(n p) d -> p n d", p=128))

            # per-batch activation stores
            qt_sb = qkv.tile([128, G, TB], bf16, tag="qt")    # [d, h, t] Q^T
            kt_sb = qkv.tile([128, TB], bf16, tag="kt")       # [d, t]    K^T
            v_sb = qkv.tile([128, NT, D], bf16, tag="v")      # [t_p, ti, d]
            at_sb = apool.tile([128, G, TB], bf16, tag="at")  # [d, h, t] attnT

            # ---- phase 1: projections + RMSNorm + RoPE + transposes ----
            for ti in range(NT):
                t0 = t0b + ti * 128
                hid_sb = hpool.tile([128, NC, 128], bf16, tag="hid")
                nc.sync.dma_start(out=hid_sb, in_=hid_v[:, :, t0:t0 + 128])

                q_ps = psA.tile([128, HD_PER_CORE], f32, tag="mm512")
                kv_ps = psB.tile([128, 2 * D], f32, tag="kvsum")
                k_ps = kv_ps[:, 0:D]
                v_ps = kv_ps[:, D:2 * D]
                for n in range(NC):
                    st, sp = (n == 0), (n == NC - 1)
                    nc.tensor.matmul(q_ps, hid_sb[:, n, :], wq_sb[:, n, :], start=st, stop=sp)
                for n in range(NC):
                    st, sp = (n == 0), (n == NC - 1)
                    nc.tensor.matmul(kv_ps, hid_sb[:, n, :], wkv_sb[:, n, :], start=st, stop=sp)

                if ti == 0:
                    load_rope_tables()

                # RMSNorm stats: ssq per head -> one batched sqrt+recip
                sq_scratch = spool.tile([128, 128], bf16, tag="sqs")
                ssq_all = spool.tile([128, G + 1], f32, tag="ssq")
                for h in range(G):
                    nc.scalar.activation(sq_scratch, q_ps[:, h * D:(h + 1) * D],
                                         mybir.ActivationFunctionType.Square,
                                         bias=zero_s[:, 0:1],
                                         accum_out=ssq_all[:, h:h + 1])
                nc.scalar.activation(sq_scratch, k_ps,
                                     mybir.ActivationFunctionType.Square,
                                     bias=zero_s[:, 0:1],
                                     accum_out=ssq_all[:, G:G + 1])
                t1a = spool.tile([128, G + 1], f32, tag="t1a")
                nc.scalar.activation(t1a, ssq_all, mybir.ActivationFunctionType.Sqrt,
                                     bias=eps_s[:, 0:1], scale=1.0 / D)
                rstd_all = spool.tile([128, G + 1], f32, tag="rsall")
                nc.vector.reciprocal_approx_fast(rstd_all, t1a)
                neg = spool.tile([128, G + 1], f32, tag="neg")
                nc.vector.tensor_scalar_mul(neg, rstd_all, -1.0)
                rstd_q = [rstd_all[:, h:h + 1] for h in range(G)]
                rs_k = rstd_all[:, G:G + 1]

                Dh = D // 2
                mul = mybir.AluOpType.mult

                def rope(src_ps, rstd, nrstd, cos_t, sin_t, out_bf):
                    a = spool.tile([128, D], bf16, tag="ra")
                    bb = spool.tile([128, D], bf16, tag="rb")
                    nc.vector.scalar_tensor_tensor(a, src_ps, rstd, cos_t, mul, mul)
                    nc.vector.scalar_tensor_tensor(bb[:, 0:Dh], src_ps[:, Dh:D],
                                                   nrstd, sin_t[:, 0:Dh], mul, mul)
                    nc.vector.scalar_tensor_tensor(bb[:, Dh:D], src_ps[:, 0:Dh],
                                                   rstd, sin_t[:, Dh:D], mul, mul)
                    nc.vector.tensor_add(out_bf, a, bb)

                qr = spool.tile([128, HD_PER_CORE], bf16, tag="qr")
                for h in range(G):
                    rope(q_ps[:, h * D:(h + 1) * D], rstd_q[h], neg[:, h:h + 1],
                         cq_sb[:, ti, :], sq_sb[:, ti, :], qr[:, h * D:(h + 1) * D])
                kr = spool.tile([128, D], bf16, tag="kr")
                rope(k_ps, rs_k, neg[:, G:G + 1], ck_sb[:, ti, :], sk_sb[:, ti, :], kr)

                # V: psum -> sbuf bf16 (both for AV lhsT and for output)
                nc.vector.tensor_copy(v_sb[:, ti, :], v_ps)
                nc.sync.dma_start(out=vnew[t0:t0 + 128, :], in_=v_sb[:, ti, :])
                nc.sync.dma_start(out=knew[t0:t0 + 128, :], in_=kr)

                # transposes into [d, t] layout on PE
                # (1/sqrt(D) scale is pre-folded into cos_q/sin_q on host)
                tp = psC.tile([128, 5 * 128], bf16, tag="accT")
                for h in range(G):
                    nc.tensor.transpose(tp[:, h * 128:(h + 1) * 128],
                                        qr[:, h * D:(h + 1) * D], ident)
                nc.tensor.transpose(tp[:, 4 * 128:5 * 128], kr, ident)
                nc.vector.tensor_copy(
                    qt_sb[:, :, ti * 128:(ti + 1) * 128],
                    tp[:, 0:4 * 128].rearrange("p (g t) -> p g t", g=G))
                nc.vector.tensor_copy(kt_sb[:, ti * 128:(ti + 1) * 128],
                                      tp[:, 4 * 128:5 * 128])

            if not wo_loaded[0]:
                nc.sync.dma_start(out=wo_sb, in_=wo_v)
                wo_loaded[0] = True

            # ---- phase 2: attention (transposed layout, exact causal) ----
            for h in range(G):
                for ci in range(4):  # query chunks of 512
                    nl = 4 * (ci + 1)  # causal l-tiles of 128
                    q_sl = qt_sb[:, h, ci * 512:(ci + 1) * 512]
                    sums = psB.tile([1, 512], f32, tag="kvsum")
                    acc = psC.tile([128, 512], f32, tag="accT")
                    for lt in range(nl):
                        # diagonal band: trim query columns that precede key lt
                        qs = max(0, (lt - 4 * ci) * 128)
                        n = 512 - qs
                        sc = psA.tile([128, 512], f32, tag="mm512")
                        nc.tensor.matmul(sc[:, 0:n], kt_sb[:, lt * 128:(lt + 1) * 128],
                                         q_sl[:, qs:512], start=True, stop=True)
                        pt = ppool.tile([128, 512], bf16, tag="pt")
                        nc.scalar.activation(pt[:, 0:n], sc[:, 0:n],
                                             mybir.ActivationFunctionType.Exp,
                                             bias=zero_s[:, 0:1])
                        if lt >= 4 * ci:  # zero strictly-future queries in diag tile
                            nc.gpsimd.affine_select(
                                pt[:, 0:n], pt[:, 0:n], pattern=[[1, n]],
                                compare_op=mybir.AluOpType.is_ge, fill=0.0,
                                base=0, channel_multiplier=-1)
                        st, sp = (lt == 0), (lt == nl - 1)
                        nc.tensor.matmul(sums[:, qs:512], ones_col, pt[:, 0:n],
                                         start=st, stop=sp)
                        nc.tensor.matmul(acc[:, qs:512], v_sb[:, lt, :], pt[:, 0:n],
                                         start=st, stop=sp)
                    rec = spool.tile([1, 512], f32, tag="rec")
                    nc.vector.reciprocal_approx_fast(rec, sums)
                    rec_bc = spool.tile([128, 512], f32, tag="recbc")
                    nc.gpsimd.partition_broadcast(rec_bc, rec)
                    nc.vector.tensor_mul(at_sb[:, h, ci * 512:(ci + 1) * 512],
                                         acc, rec_bc)

            # ---- phase 3: o_proj partials, DMA straight from PSUM ----
            for ti in range(NT):
                t0 = t0b + ti * 128
                for ec in range(HID // 512):
                    op = psA.tile([128, 512], f32, tag="mm512")
                    for h in range(G):
                        nc.tensor.matmul(op, at_sb[:, h, ti * 128:(ti + 1) * 128],
                                         wo_sb[:, h, ec * 512:(ec + 1) * 512],
                                         start=(h == 0), stop=(h == G - 1))
                    ob = ppool.tile([128, 512], bf16, tag="ob")
                    nc.vector.tensor_copy(ob, op)
                    nc.sync.dma_start(out=out_p[t0:t0 + 128, ec * 512:(ec + 1) * 512],
                                      in_=ob)

    nc.compile()
    return nc


def kernel(hidden, cos, sin, wq, wk, wv, wo, q_gamma, k_gamma):
    global _COMPILED
    if _COMPILED is None:
        _COMPILED = _build()
    nc = _COMPILED

    hid_t = np.ascontiguousarray(hidden.reshape(T, HID).T).astype(BF16)
    wq_t = wq.T.astype(BF16)   # (HID, H*D)
    wk_t = wk.T.astype(BF16)
    wv_t = wv.T.astype(BF16)
    # merged K|V projection weights, per-core slice stacked on output dim
    wo_t = np.ascontiguousarray(wo.T)  # (H*D, HID)

    cos2 = cos.reshape(T, D).astype(np.float32)
    sin2 = sin.reshape(T, D).astype(np.float32)
    qg = q_gamma.astype(np.float32)
    kg = k_gamma.astype(np.float32)
    qg_sw = np.concatenate([qg[D // 2:], qg[:D // 2]])
    kg_sw = np.concatenate([kg[D // 2:], kg[:D // 2]])
    cos_q = (cos2 * qg * SCALE).astype(BF16)
    sin_q = (sin2 * qg_sw * SCALE).astype(BF16)
    cos_k = (cos2 * kg).astype(BF16)
    sin_k = (sin2 * kg_sw).astype(BF16)

    in_maps = []
    for c in range(NCORES):
        in_maps.append({
            "hid_t": hid_t,
            "wq_t": np.ascontiguousarray(wq_t[:, c * HD_PER_CORE:(c + 1) * HD_PER_CORE]),
            "wkv_t": np.ascontiguousarray(np.concatenate(
                [wk_t[:, c * D:(c + 1) * D], wv_t[:, c * D:(c + 1) * D]], axis=1)),
            "wo_t": np.ascontiguousarray(wo_t[c * HD_PER_CORE:(c + 1) * HD_PER_CORE, :]).astype(BF16),
            "cos_q": cos_q, "sin_q": sin_q, "cos_k": cos_k, "sin_k": sin_k,
        })

    res = bass_utils.run_bass_kernel_spmd(nc, in_maps, core_ids=list(range(NCORES)))
    outs = res.results

    out = np.zeros((T, HID), dtype=np.float32)
    for c in range(NCORES):
        out += np.asarray(outs[c]["out_p"], dtype=np.float32)
    out = out.reshape(B, S, HID)

    new_k = np.stack(
        [np.asarray(outs[c]["knew"]).astype(np.float32).reshape(B, S, D)
         for c in range(NCORES)], axis=1)
    new_v = np.stack(
        [np.asarray(outs[c]["vnew"]).astype(np.float32).reshape(B, S, D)
         for c in range(NCORES)], axis=1)
    return out, new_k, new_v
